# revision 26
# baseline (speedup 1.0000x reference)
"""NeighborSample Trainium2 kernel, v18: all-c32 3-queue balanced schedule.

Input  x:   (8, 64, 64, 192) f32
Output:     (8*64*64, 5, 5, 192) f32 — out[b*4096 + h*64 + w, i, j, c] =
            x[b, h+i-2, w+j-2, c] (zero-padded).

Pure DMA, data-parallel over batch (1 sample per NeuronCore). Input is
zero-padded on the host to (68, 68, 192).

HW model (measured via ntff profiles of v16-v23):
- The 16 SDMA engines (E64..E79) are a shared ~425 GB/s ceiling
  (~26.5 GB/s each; store slice rate 25.7 GB/s at 3840 B
  descriptors). Two busy queues already saturate it; per-queue rate is
  a demand-weighted share (SWDGE's 2-desc packets get a 2:1:1
  round-robin share = 212 GB/s vs the two HWDGE rings).
- Every dma_start MUST carry sync info; its per-engine sem-update
  descriptor is a WAW fence (engine stalls until writes ack). Fence
  count == DMA count: 14 DMAs -> 219 us (this file); 19 -> 256 us;
  25 -> 313 us. balance_dma_aps rejects >3-dim APs, so shifts cannot
  be merged into fewer DMAs.
- SWDGE (gpsimd, Q0) first transfer lands ~15 us in (desc-gen)
  regardless of DMA size; sustains ~212 GB/s under contention, ~370
  alone.
- DRAM->DRAM counts once against the engine ceiling but twice on HBM
  (~716 GB/s): one queue of D2D (212 GB/s) + all stores ≈ 650 GB/s
  fits; any more D2D throttles on HBM.
- Loads (DRAM->SBUF) are read-latency-bound at ~7-8 GB/s/engine/queue.
- Stores with <16 engine slots strand capacity once that queue's other
  engines run dry (v17/v22 regressions); keep every store c32
  (16 slots x 2 rows). Per-engine byte skew surfaces as a tail
  straggle: the c2 load costs E64/65 ~+5 us; routing it via Q0
  deadlocked/degraded the device (v23) — keep it on the rings.

Total SDMA bytes 82.2 MB -> ~194 us floor; measured 219 us =
preamble 4.5 + load-phase ramp ~12 + plateau 424.6 GB/s + tail ~9.

Layout (10 c32 stores = 2 halves x 5 shifts):
- gpsimd/SWDGE: i=0 and i=4 for both halves, DRAM->DRAM from padded x
  (zero deps, 31.4 MB; drains ~t=178, rings then sprint at 210 each).
- sync (h0) / scalar (h1): load rows 1-32 (c32) + rows 33-34 (c2,
  dedicated sems), then stores i=1 (gate la1), i=2, i=3 (gate la3).
  25.4 MB each. No cross-engine dependencies anywhere.

DMA fan-out rule (measured): outermost count c splits over n = (largest
divisor of c <= 16) engine slots, c/n consecutive rows per slot. Sem
increments total +16 per DMA; a dedicated sem per gating load (a wait
can otherwise be satisfied by another DMA's increments).
"""

import sys

for _p in ("/opt/trn_rl_repo",):
    if _p not in sys.path:
        sys.path.insert(0, _p)

import numpy as np

import concourse.bass as bass
import concourse.mybir as mybir
from concourse.bass_utils import run_bass_kernel_spmd

B = 8
H = W = 64
C = 192
K = 5
PAD = 2
HP = H + 2 * PAD     # 68 padded rows
WP = W + 2 * PAD     # 68 padded cols
ROW = WP * C         # 13056 elems per partition (one padded row)
WIN = K * C          # 960: one (h, w, i) output chunk
OUT_W = K * K * C    # 4800
OUT_H = W * OUT_W    # 307200
HH = H // 2          # 32 output rows per half


def _store(eng, out, buf, half, i, w0=0, wcnt=W):
    """SBUF->DRAM store: shift i, all 32 rows of this half, w[w0:w0+wcnt)."""
    return eng.dma_start(
        out=bass.AP(
            out,
            (HH * half) * OUT_H + i * WIN + w0 * OUT_W,
            [[OUT_H, HH], [OUT_W, wcnt], [1, WIN]],
        ),
        in_=bass.AP(
            buf,
            (64 * half + i) * ROW + w0 * C,
            [[ROW, HH], [C, wcnt], [1, WIN]],
        ),
    )


def _load(eng, x, buf, half, r0, cnt):
    """Load this half's padded rows [r0, r0+cnt) into partitions."""
    return eng.dma_start(
        out=bass.AP(buf, (64 * half + r0) * ROW, [[ROW, cnt], [1, ROW]]),
        in_=bass.AP(x, (HH * half + r0) * ROW, [[ROW, cnt], [1, ROW]]),
    )


def _dram_store(eng, x, out, half, i, w0=0, wcnt=W):
    """Shift i, w[w0:w0+wcnt), direct from padded x in DRAM — no SBUF,
    no deps."""
    return eng.dma_start(
        out=bass.AP(
            out,
            (HH * half) * OUT_H + i * WIN + w0 * OUT_W,
            [[OUT_H, HH], [OUT_W, wcnt], [1, WIN]],
        ),
        in_=bass.AP(
            x,
            (HH * half + i) * ROW + w0 * C,
            [[ROW, HH], [C, wcnt], [1, WIN]],
        ),
    )


def _emit_ring(eng, x, out, buf, la1, la3, dsem, half):
    # Loads MUST come first on the ring: a dependency-free D2D filler
    # ahead of them (v24) lifted t=10-15 us to 371 GB/s but pushed la1
    # to ~40 us, starving both rings mid-ramp (223 us total, +4).
    _load(eng, x, buf, half, 1, 32).then_inc(la1, 16)
    _load(eng, x, buf, half, 33, 2).then_inc(la3, 16)
    eng.wait_ge(la1, 16)
    _store(eng, out, buf, half, 1).then_inc(dsem, 16)
    eng.wait_ge(la3, 16)
    _store(eng, out, buf, half, 2).then_inc(dsem, 16)
    _store(eng, out, buf, half, 3).then_inc(dsem, 16)
    eng.wait_ge(dsem, 16 * 3)


def build_nc() -> bass.Bass:
    nc = bass.Bass()
    x = nc.declare_dram_parameter("x", [HP, WP, C], mybir.dt.float32, isOutput=False)
    out = nc.declare_dram_parameter(
        "out", [H, W, K, K, C], mybir.dt.float32, isOutput=True
    )

    with (
        nc.Block() as block,
        nc.semaphore("la1") as la1,
        nc.semaphore("la3") as la3,
        nc.semaphore("d_a") as d_a,
        nc.semaphore("lb1") as lb1,
        nc.semaphore("lb3") as lb3,
        nc.semaphore("d_b") as d_b,
        nc.semaphore("d_g") as d_g,
        nc.sbuf_tensor("buf", [128, ROW], mybir.dt.float32) as buf,
    ):

        @block.sync
        def _(sync):
            _emit_ring(sync, x, out, buf, la1, la3, d_a, 0)

        @block.scalar
        def _(scalar):
            _emit_ring(scalar, x, out, buf, lb1, lb3, d_b, 1)

        @block.gpsimd
        def _(gpsimd):
            _dram_store(gpsimd, x, out, 0, 0).then_inc(d_g, 16)
            _dram_store(gpsimd, x, out, 1, 0).then_inc(d_g, 16)
            _dram_store(gpsimd, x, out, 0, 4).then_inc(d_g, 16)
            _dram_store(gpsimd, x, out, 1, 4).then_inc(d_g, 16)
            gpsimd.wait_ge(d_g, 16 * 4)

    return nc


_NC_CACHE = None


def prep_in_maps(x):
    xp = np.zeros((B, HP, WP, C), dtype=np.float32)
    xp[:, PAD : PAD + H, PAD : PAD + W, :] = x
    return [{"x": np.ascontiguousarray(xp[i])} for i in range(B)]


def kernel(x) -> np.ndarray:
    global _NC_CACHE
    x = np.asarray(x, dtype=np.float32)
    assert x.shape == (B, H, W, C), x.shape
    if _NC_CACHE is None:
        _NC_CACHE = build_nc()
    in_maps = prep_in_maps(x)
    res = run_bass_kernel_spmd(_NC_CACHE, in_maps, list(range(B)))
    outs = [res.results[i]["out"].reshape(H * W, K, K, C) for i in range(B)]
    return np.concatenate(outs, axis=0)
